# revision 43
# baseline (speedup 1.0000x reference)
"""Multi-head attention (B=2, L=2048, DIM=2048, H=16) on 8 TRN2 NeuronCores.

Sharding: data-parallel over batch (2) x tensor-parallel over head groups (4).
Core c handles batch c//4, heads [4*(c%4), 4*(c%4)+4): it receives the full
query/value tensors for its batch plus the 512-row slices of Wq/Wk/Wv for its
heads, and produces the [2048, 512] slice of the output.

Schedule, designed so the PE never starves and the Act engine's exp work
(~133us) hides under projection matmuls:
  startup: xv staged immediately; x tiles are DMA'd f32r then converted to
           bf16 right away (bf16 PE transposes run 1 cyc/row).
  phase 0: Wk/Wv transposed up front (staged as halves so the first
           transpose starts ~3us in); Wq^T is emitted inside phase 1
           (reusing the V-projection PSUM tag) since it's needed much later.
  phase 1: K/V projection over 8 l-blocks; K accumulated 4-heads-per-tile.
  phase 2: per 512-query block, a software pipeline interleaved at k2
           (1k-score-tile) granularity: the pending head's attn*V chains
           and one piece of the NEXT block's Q projection are emitted
           inside the exp-paced score loop so the PE stays saturated while
           Act does exp. The query mask is folded into the Q drain via
           tensor_mul with a broadcast mask tile; softmax denominators come
           from a ones-column appended to V; per-qs drains and out-DMAs
           are emitted as soon as each accumulation chain stops.
PSUM drains only on Act/DVE (hardware forbids GPSIMD PSUM access); GpSimd
does bf16 converts and part of the DMA issue. Pool scopes nest LIFO for
the stack allocator. CoreSim cost model: ~347us; PE busy 91.5% (318us of
PE work at 2.4 GHz: 164us QKV matmuls, 110us scores+attn*V, 44us
transposes).
"""

import sys

for p in ("/opt/trn_rl_repo", "/opt/pypackages"):
    if p not in sys.path:
        sys.path.insert(0, p)

import contextlib
import itertools as it

import numpy as np

import concourse.bacc as bacc
import concourse.mybir as mybir
import concourse.tile as tile
from concourse import masks
from concourse.bass_utils import run_bass_kernel_spmd

N_CORES = 8
B, L, DIM, H = 2, 2048, 2048, 16
JB = DIM // 4          # 512 output features per core (4 heads)
DH = 128               # head dim
NH = 4                 # heads per core
NDC = DIM // 128       # 16 contraction chunks
LB = 256               # l-block for QKV staging
NLB = L // LB          # 8
NKT = L // 128         # 16 k tiles
QB = 512               # q-block for attention
NQB = L // QB          # 4
HD = DIM // 2
SCALE = 1.0 / np.sqrt(DH)

F32 = mybir.dt.float32
F32R = mybir.dt.float32r
BF16 = mybir.dt.bfloat16


def drain(nc, eng, out_ap, in_ap):
    """PSUM->SBUF copy on the named engine (act or dve only)."""
    if eng == "act":
        nc.scalar.copy(out_ap, in_ap)
    else:
        nc.vector.tensor_copy(out_ap, in_ap)


def build_nc():
    nc = bacc.Bacc("TRN2", target_bir_lowering=False, debug=False,
                   num_devices=N_CORES)
    xq = nc.dram_tensor("xq", [L, DIM], F32R, kind="ExternalInput").ap()
    xv = nc.dram_tensor("xv", [L, DIM], F32R, kind="ExternalInput").ap()
    msk = nc.dram_tensor("msk", [1, L], F32, kind="ExternalInput").ap()
    wq = nc.dram_tensor("wq", [JB, DIM], F32R, kind="ExternalInput").ap()
    wk = nc.dram_tensor("wk", [JB, DIM], F32R, kind="ExternalInput").ap()
    wv = nc.dram_tensor("wv", [JB, DIM], F32R, kind="ExternalInput").ap()
    out = nc.dram_tensor("out", [L, JB], F32, kind="ExternalOutput").ap()

    with tile.TileContext(nc) as tc:
        build_kernel(nc, tc, xq, xv, msk, wq, wk, wv, out)
    nc.compile()
    return nc


def build_kernel(nc, tc, xq, xv, msk, wq, wk, wv, out):
    octx = contextlib.ExitStack()
    with octx:
        # ---- persistent pools (live for the whole kernel) ----
        const = octx.enter_context(tc.tile_pool(name="const", bufs=1))
        identf = const.tile([128, 128], F32, name="identf")
        ident_r = const.tile([128, 128], F32R, name="ident_r")
        ident_b = const.tile([128, 128], BF16, name="ident_b")
        mask_bc = const.tile([128, L], F32, name="mask_bc")
        m1 = const.tile([1, L], F32, name="m1")

        ktp = octx.enter_context(tc.tile_pool(name="ktp", bufs=1))
        K_T = [ktp.tile([128, L], BF16, tag=f"k{h}", name=f"kT{h}")
               for h in range(NH)]
        vp = octx.enter_context(tc.tile_pool(name="vp", bufs=1))
        V = [vp.tile([128, NH * 129], BF16, tag=f"v{t}", name=f"vS{t}")
             for t in range(NKT)]
        wqt_pool = octx.enter_context(tc.tile_pool(name="wqt", bufs=1))
        wq_t = [wqt_pool.tile([128, JB], BF16, tag=f"wq{dc}", name=f"wqT{dc}")
                for dc in range(NDC)]
        xst = octx.enter_context(tc.tile_pool(name="xst", bufs=1))
        xbp = octx.enter_context(tc.tile_pool(name="xbp", bufs=2))
        xtp = octx.enter_context(tc.tile_pool(name="xtp", bufs=2))

        # identities (DVE) — emitted first so DVE is free for W drains later
        masks.make_identity(nc, identf[:])
        nc.vector.tensor_copy(ident_r[:], identf[:])
        nc.vector.tensor_copy(ident_b[:], identf[:])

        stage = {}

        def stage_dma(x_dram, ltile, key, conv):
            """DMA one [128, DIM] l-tile (f32r), convert to bf16 at once."""
            xs = xst.tile([128, DIM], F32R, tag=f"x{ltile % 2}",
                          name=f"x{ltile % 2}")
            nc.sync.dma_start(xs[:], x_dram[ltile * 128:(ltile + 1) * 128, :])
            xb = xbp.tile([128, DIM], BF16, tag=f"xb{ltile % 4}",
                          name=f"xb{ltile % 4}")
            if conv == "pool":
                nc.gpsimd.tensor_copy(xb[:], xs[:])
            else:
                nc.vector.tensor_copy(xb[:], xs[:])
            stage[key] = xb

        def transpose_x(xb_pair, pps, dr_cycle):
            """Transpose two converted l-tiles into 16 xt tiles [128, LB],
            two dc per PSUM tile to halve allocation pressure."""
            xt = []
            for dp in range(NDC // 2):
                pt = pps.tile([128, 2 * LB], BF16, tag="tp", name="tp")
                for j in range(2):
                    dc = 2 * dp + j
                    for lt in range(2):
                        nc.tensor.transpose(
                            pt[:, j * LB + lt * 128:j * LB + (lt + 1) * 128],
                            xb_pair[lt][:, dc * 128:(dc + 1) * 128],
                            ident_b[:])
                for j in range(2):
                    dc = 2 * dp + j
                    t = xtp.tile([128, LB], BF16, tag=f"xt{dc}",
                                 name=f"xt{dc}")
                    drain(nc, next(dr_cycle), t[:],
                          pt[:, j * LB:(j + 1) * LB])
                    xt.append(t)
            return xt

        # stage lb0 of xv right away (sync queue; DVE converts since the
        # Pool queue is saturated issuing W DMAs at startup)
        for tl in range(2):
            stage_dma(xv, tl, tl, "dve")
        nc.sync.dma_start(m1[:], msk[:, :])

        # ---- phases 0+1 scope: wk_t/wv_t ----
        with tc.tile_pool(name="wkvt", bufs=1) as wkvt_pool:
            wk_t = [wkvt_pool.tile([128, JB], BF16, tag=f"wk{dc}",
                                   name=f"wkT{dc}") for dc in range(NDC)]
            wv_t = [wkvt_pool.tile([128, JB], BF16, tag=f"wv{dc}",
                                   name=f"wvT{dc}") for dc in range(NDC)]
            wdr = it.cycle(["dve", "dve", "act"])
            # phase 0: Wk/Wv halves in, transpose (f32r: 1.5 cyc/row)
            with tc.tile_pool(name="wst", bufs=1) as wst, \
                 tc.tile_pool(name="wps", bufs=4, space="PSUM") as wps:
                engines = (nc.scalar, nc.gpsimd)
                for w_dram, dst, nm in ((wk, wk_t, "wk"), (wv, wv_t, "wv")):
                    rows = [[None, None] for _ in range(4)]
                    for hf in range(2):      # all first halves land first
                        for i in range(4):
                            r = wst.tile([128, HD], F32R, tag=f"w{i}h{hf}",
                                         name=f"{nm}r{i}h{hf}")
                            engines[i % 2].dma_start(
                                r[:], w_dram[i * 128:(i + 1) * 128,
                                             hf * HD:(hf + 1) * HD])
                            rows[i][hf] = r
                    for dc in range(NDC):
                        hf, dco = divmod(dc, NDC // 2)
                        pt = wps.tile([128, JB], F32R, tag="wp", name="wp")
                        for i in range(4):
                            nc.tensor.transpose(
                                pt[:, i * 128:(i + 1) * 128],
                                rows[i][hf][:, dco * 128:(dco + 1) * 128],
                                ident_r[:])
                        drain(nc, next(wdr), dst[dc][:], pt[:])

            # ---- phase 1: K/V projection (+ Wq^T after lb0) ----
            with tc.tile_pool(name="wqst", bufs=2) as wqst, \
                 tc.tile_pool(name="p1ps", bufs=2, space="PSUM") as pps, \
                 tc.tile_pool(name="p1ka", bufs=2, space="PSUM") as kps, \
                 tc.tile_pool(name="p1va", bufs=2, space="PSUM") as vps:
                dr_xt1 = it.cycle(["act", "act", "dve"])
                for lb in range(NLB):
                    if lb == 0:
                        for tl in range(2, 4):
                            stage_dma(xv, tl, tl, "dve")
                    elif lb == 1:
                        # mask broadcast (gpsimd) — needed only in phase 2,
                        # deferred so it doesn't block startup converts
                        nc.gpsimd.partition_broadcast(mask_bc[:], m1[:])
                    xt = transpose_x(
                        [stage.pop(2 * lb), stage.pop(2 * lb + 1)],
                        pps, dr_xt1)
                    # K: 4 head-chunks chained into one 2-bank PSUM tile
                    ka = kps.tile([128, 4 * LB], F32, tag="ka", name="ka")
                    for jt in range(NH):
                        for dc in range(NDC):
                            nc.tensor.matmul(
                                ka[:, jt * LB:(jt + 1) * LB],
                                wk_t[dc][:, jt * 128:(jt + 1) * 128],
                                xt[dc][:],
                                start=(dc == 0), stop=(dc == NDC - 1))
                    for jt in range(NH):
                        drain(nc, "dve", K_T[jt][:, lb * LB:(lb + 1) * LB],
                              ka[:, jt * LB:(jt + 1) * LB])
                    if lb == 0:
                        # V ones columns (DVE; must precede lb0's V drains)
                        for t in range(NKT):
                            nc.vector.memset(V[t][:], 1.0)
                        # stage Wq rows (gpsimd queue; Act's queue must stay
                        # clear for drains)
                        wq_rows = []
                        for i in range(4):
                            r = wqst.tile([128, DIM], F32R, tag=f"q{i % 2}",
                                          name=f"wqr{i}")
                            nc.gpsimd.dma_start(
                                r[:], wq[i * 128:(i + 1) * 128, :])
                            wq_rows.append(r)
                    # V: seq-major with ones column per head
                    for lt in range(2):
                        va = vps.tile([128, JB], F32, tag="va", name="va")
                        for dc in range(NDC):
                            nc.tensor.matmul(
                                va[:], xt[dc][:, lt * 128:(lt + 1) * 128],
                                wv_t[dc][:],
                                start=(dc == 0), stop=(dc == NDC - 1))
                        kt = lb * 2 + lt
                        for h in range(NH):
                            drain(nc, "dve",
                                  V[kt][:, h * 129:h * 129 + 128],
                                  va[:, h * 128:(h + 1) * 128])
                    if lb == 0:
                        # Wq^T between lb0 and lb1, PSUM via the "va" tag
                        for dc in range(NDC):
                            pt = vps.tile([128, JB], F32R, tag="va",
                                          name="wqp")
                            for i in range(4):
                                nc.tensor.transpose(
                                    pt[:, i * 128:(i + 1) * 128],
                                    wq_rows[i][:, dc * 128:(dc + 1) * 128],
                                    ident_r[:])
                            drain(nc, next(wdr), wq_t[dc][:], pt[:])
                    # prefetch next lb (xv); tail feeds phase-2 qb0 (xq)
                    if lb < NLB - 1:
                        for lt in range(2):
                            tl = 2 * (lb + 1) + lt
                            if tl >= 4:
                                stage_dma(xv, tl, tl, "pool")
                    if lb == NLB - 2:
                        for lt in range(2):
                            stage_dma(xq, lt, 100 + lt, "pool")
                    elif lb == NLB - 1:
                        for lt in range(2):
                            stage_dma(xq, 2 + lt, 102 + lt, "pool")

        # ---- phase 2: Q projection pipelined with attention ----
        with tc.tile_pool(name="qtp", bufs=2) as qtp, \
             tc.tile_pool(name="et", bufs=2) as etp, \
             tc.tile_pool(name="ot", bufs=1) as otp, \
             tc.tile_pool(name="rsb", bufs=4) as rsb, \
             tc.tile_pool(name="p2tp", bufs=2, space="PSUM") as pps2, \
             tc.tile_pool(name="p2s", bufs=2, space="PSUM") as sps, \
             tc.tile_pool(name="p2a", bufs=1, space="PSUM") as aps:
            dr_xt2 = it.cycle(["dve"])
            QT = {}           # qb -> 4 tiles [128, QB]
            ots = {}          # qb -> 4 tiles [128, JB]
            ET = {}           # (qb, h) -> 8 et tiles

            def qproj_chunk(nqb, ci):
                """Emit chunk ci (0..3) of Q-projection for block nqb
                (monolithic form, used by the prologue)."""
                for piece in qproj_pieces(nqb, ci):
                    piece()

            def qproj_pieces(nqb, ci):
                """Chunk ci (0..3) of Q-projection for nqb as 8 closures of
                ~k2-sized PE work, for interleaving into the exp-paced
                attention loop."""
                if ci == 0:
                    QT[nqb] = [qtp.tile([128, QB], BF16, tag=f"q{h}",
                                        name=f"qT{h}") for h in range(NH)]
                if ci in (0, 2):
                    i = ci // 2
                    pair = [stage.pop(100 + 4 * nqb + 2 * i + lt)
                            for lt in range(2)]
                    xt = stage[(nqb, i)] = []

                    def tp_piece(p):
                        pt = pps2.tile([128, 2 * LB], BF16, tag="tp",
                                       name="tp")
                        for j in range(2):
                            dc = 2 * p + j
                            for lt in range(2):
                                nc.tensor.transpose(
                                    pt[:, j * LB + lt * 128:
                                       j * LB + (lt + 1) * 128],
                                    pair[lt][:, dc * 128:(dc + 1) * 128],
                                    ident_b[:])
                        for j in range(2):
                            dc = 2 * p + j
                            t = xtp.tile([128, LB], BF16, tag=f"xt{dc}",
                                         name=f"xt{dc}")
                            drain(nc, next(dr_xt2), t[:],
                                  pt[:, j * LB:(j + 1) * LB])
                            xt.append(t)
                    return [lambda p=p: tp_piece(p) for p in range(8)]
                i = (ci - 1) // 2
                xt = stage[(nqb, i)]
                box = {}

                def mm_piece(p):
                    jt, hf = divmod(p, 2)
                    if p == 0:
                        box[0] = sps.tile([128, 4 * LB], F32, tag="s",
                                          name="qa")
                        stage.pop((nqb, i), None)
                    qa = box[0]
                    col = jt * LB
                    for dc in range(hf * 8, hf * 8 + 8):
                        nc.tensor.matmul(
                            qa[:, col:col + LB],
                            wq_t[dc][:, jt * 128:(jt + 1) * 128],
                            xt[dc][:],
                            start=(dc == 0), stop=(dc == NDC - 1))
                    if hf == 1:
                        nc.vector.tensor_mul(
                            QT[nqb][jt][:, i * LB:(i + 1) * LB],
                            qa[:, col:col + LB],
                            mask_bc[:, nqb * QB + i * LB:
                                    nqb * QB + (i + 1) * LB])
                return [lambda p=p: mm_piece(p) for p in range(8)]

            def attn_chains(qb_h, a01, a23, k2):
                """8 attn*V matmuls for the pending head: chain qs=k2//2,
                half k2%2 — each qs chain spans 2 consecutive k2 steps so
                chains never interleave within a PSUM zero region. The qs
                drain is emitted as soon as its chain stops, freeing the
                PSUM bank for the next head without waiting for slot end."""
                pqb, ph = qb_h
                ets = ET[(pqb, ph)]
                qs, half = divmod(k2, 2)
                a = a01 if qs < 2 else a23
                base = 129 * (qs % 2)
                for kc in range(half * 8, half * 8 + 8):
                    nc.tensor.matmul(
                        a[:, base:base + 129],
                        ets[kc // 2][:, (kc % 2) * 512
                                     + qs * 128:(kc % 2) * 512
                                     + (qs + 1) * 128],
                        V[kc][:, ph * 129:(ph + 1) * 129],
                        start=(kc == 0), stop=(kc == NKT - 1))
                if half == 1:
                    if pqb not in ots:
                        ots[pqb] = [otp.tile([128, JB], F32, tag=f"ot{q}",
                                             name=f"ot{q}") for q in range(4)]
                    rec = rsb.tile([128, 1], F32, tag="rec", name="rec")
                    nc.vector.reciprocal(rec[:], a[:, base + 128:base + 129])
                    nc.vector.tensor_scalar_mul(
                        ots[pqb][qs][:, ph * 128:(ph + 1) * 128],
                        a[:, base:base + 128], rec[:])
                    if ph == NH - 1:
                        q0 = pqb * QB + qs * 128
                        nc.sync.dma_start(out[q0:q0 + 128, :],
                                          ots[pqb][qs][:])

            def attn_drain(qb_h, a01, a23):
                """Slot-end bookkeeping."""
                del ET[qb_h]

            # prologue: stage qb1's tiles, project qb 0
            for t in range(4):
                stage_dma(xq, 4 + t, 104 + t, "pool")
            for ci in range(4):
                qproj_chunk(0, ci)

            pending = None
            pend_a = None
            for qb in range(NQB):
                for h in range(NH):
                    if pending is not None:
                        a01 = aps.tile([128, 258], F32, tag="a01", name="a01")
                        a23 = aps.tile([128, 258], F32, tag="a23", name="a23")
                        pend_a = (a01, a23)
                    ET[(qb, h)] = [etp.tile([128, 1024], BF16, tag=f"et{k2}",
                                            name=f"et{k2}")
                                   for k2 in range(NKT // 2)]
                    pieces = (qproj_pieces(qb + 1, h)
                              if qb < NQB - 1 else [None] * 8)
                    for k2 in range(NKT // 2):
                        # pending head's chains and proj pieces first: both
                        # are ready to run, while the s-tile allocation
                        # below can stall on exp throughput
                        if pending is not None:
                            attn_chains(pending, *pend_a, k2)
                        if pieces[k2] is not None:
                            pieces[k2]()
                        s = sps.tile([128, 1024], F32, tag="s", name="s")
                        for i in range(2):
                            nc.tensor.matmul(
                                s[:, i * 512:(i + 1) * 512],
                                K_T[h][:, (2 * k2 + i) * 128:
                                       (2 * k2 + i + 1) * 128],
                                QT[qb][h][:],
                                start=True, stop=True)
                        nc.scalar.activation(
                            ET[(qb, h)][k2][:], s[:],
                            mybir.ActivationFunctionType.Exp,
                            scale=float(SCALE))
                    if pending is not None:
                        attn_drain(pending, *pend_a)
                    if h == NH - 1 and qb < NQB - 2:
                        for t in range(4):
                            tl = 4 * (qb + 2) + t
                            stage_dma(xq, tl, 100 + tl, "pool")
                    pending = (qb, h)
                del QT[qb]
            # tail: last head's chains
            a01 = aps.tile([128, 258], F32, tag="a01", name="a01")
            a23 = aps.tile([128, 258], F32, tag="a23", name="a23")
            for k2 in range(NKT // 2):
                attn_chains(pending, a01, a23, k2)
            attn_drain(pending, a01, a23)


_NC_CACHE = None


def _get_nc():
    global _NC_CACHE
    if _NC_CACHE is None:
        _NC_CACHE = build_nc()
    return _NC_CACHE


def make_in_maps(query_tensor, value_tensor, attention_mask, Wq, Wk, Wv):
    in_maps = []
    for c in range(N_CORES):
        b, g = divmod(c, 4)
        j0 = g * JB
        in_maps.append({
            "xq": np.ascontiguousarray(query_tensor[b], dtype=np.float32),
            "xv": np.ascontiguousarray(value_tensor[b], dtype=np.float32),
            "msk": np.ascontiguousarray(
                attention_mask[b].reshape(1, L), dtype=np.float32),
            "wq": np.ascontiguousarray(Wq[j0:j0 + JB], dtype=np.float32),
            "wk": np.ascontiguousarray(Wk[j0:j0 + JB], dtype=np.float32),
            "wv": np.ascontiguousarray(Wv[j0:j0 + JB], dtype=np.float32),
        })
    return in_maps


def assemble(results):
    out = np.empty((B, L, DIM), dtype=np.float32)
    for c in range(N_CORES):
        b, g = divmod(c, 4)
        out[b, :, g * JB:(g + 1) * JB] = results[c]["out"]
    return out


def kernel(query_tensor, value_tensor, attention_mask, Wq, Wk, Wv):
    nc = _get_nc()
    in_maps = make_in_maps(np.asarray(query_tensor), np.asarray(value_tensor),
                           np.asarray(attention_mask), np.asarray(Wq),
                           np.asarray(Wk), np.asarray(Wv))
    res = run_bass_kernel_spmd(nc, in_maps, core_ids=list(range(N_CORES)))
    return assemble(res.results)


# revision 46
# speedup vs baseline: 1.3815x; 1.3815x over previous
"""Multi-head attention (B=2, L=2048, DIM=2048, H=16) on 8 TRN2 NeuronCores.

Sharding: data-parallel over batch (2) x tensor-parallel over head groups (4).
Core c handles batch c//4, heads [4*(c%4), 4*(c%4)+4): it receives the full
query/value tensors for its batch plus the 512-row slices of Wq/Wk/Wv for its
heads, and produces the [2048, 512] slice of the output.

Schedule, designed so the PE never starves and the Act engine's exp work
(~133us) hides under projection matmuls:
  startup: xv staged immediately; x tiles are DMA'd f32r then converted to
           bf16 right away (bf16 PE transposes run 1 cyc/row).
  phase 0: Wk/Wv transposed up front (staged as halves so the first
           transpose starts ~3us in); Wq^T is emitted inside phase 1
           (reusing the V-projection PSUM tag) since it's needed much later.
  phase 1: K/V projection over 8 l-blocks; K accumulated 4-heads-per-tile.
  phase 2: per 512-query block, a software pipeline interleaved at k2
           (1k-score-tile) granularity: the pending head's attn*V chains
           and one piece of the NEXT block's Q projection are emitted
           inside the exp-paced score loop so the PE stays saturated while
           Act does exp. The query mask is folded into the Q drain via
           tensor_mul with a broadcast mask tile; softmax denominators come
           from a ones-column appended to V; per-qs drains and out-DMAs
           are emitted as soon as each accumulation chain stops.
PSUM drains only on Act/DVE (hardware forbids GPSIMD PSUM access); GpSimd
does bf16 converts and part of the DMA issue. Pool scopes nest LIFO for
the stack allocator. CoreSim cost model: ~347us; PE busy 91.5% (318us of
PE work at 2.4 GHz: 164us QKV matmuls, 110us scores+attn*V, 44us
transposes).
"""

import sys

for p in ("/opt/trn_rl_repo", "/opt/pypackages"):
    if p not in sys.path:
        sys.path.insert(0, p)

import contextlib
import itertools as it

import numpy as np

import concourse.bacc as bacc
import concourse.mybir as mybir
import concourse.tile as tile
from concourse import masks
from concourse.bass_utils import run_bass_kernel_spmd

N_CORES = 8
B, L, DIM, H = 2, 2048, 2048, 16
JB = DIM // 4          # 512 output features per core (4 heads)
DH = 128               # head dim
NH = 4                 # heads per core
NDC = DIM // 128       # 16 contraction chunks
LB = 256               # l-block for QKV staging
NLB = L // LB          # 8
NKT = L // 128         # 16 k tiles
QB = 512               # q-block for attention
NQB = L // QB          # 4
HD = DIM // 2
SCALE = 1.0 / np.sqrt(DH)

F32 = mybir.dt.float32
F32R = mybir.dt.float32r
BF16 = mybir.dt.bfloat16


def drain(nc, eng, out_ap, in_ap):
    """PSUM->SBUF copy on the named engine (act or dve only)."""
    if eng == "act":
        nc.scalar.copy(out_ap, in_ap)
    else:
        nc.vector.tensor_copy(out_ap, in_ap)


def build_nc():
    nc = bacc.Bacc("TRN2", target_bir_lowering=False, debug=False,
                   num_devices=N_CORES)
    xq = nc.dram_tensor("xq", [L, DIM], F32R, kind="ExternalInput").ap()
    xv = nc.dram_tensor("xv", [L, DIM], F32R, kind="ExternalInput").ap()
    msk = nc.dram_tensor("msk", [1, L], F32, kind="ExternalInput").ap()
    wq = nc.dram_tensor("wq", [JB, DIM], F32R, kind="ExternalInput").ap()
    wk = nc.dram_tensor("wk", [JB, DIM], F32R, kind="ExternalInput").ap()
    wv = nc.dram_tensor("wv", [JB, DIM], F32R, kind="ExternalInput").ap()
    out = nc.dram_tensor("out", [L, JB], F32, kind="ExternalOutput").ap()

    with tile.TileContext(nc) as tc:
        build_kernel(nc, tc, xq, xv, msk, wq, wk, wv, out)
    nc.compile()
    return nc


def build_kernel(nc, tc, xq, xv, msk, wq, wk, wv, out):
    octx = contextlib.ExitStack()
    with octx:
        # ---- persistent pools (live for the whole kernel) ----
        const = octx.enter_context(tc.tile_pool(name="const", bufs=1))
        identf = const.tile([128, 128], F32, name="identf")
        ident_r = const.tile([128, 128], F32R, name="ident_r")
        ident_b = const.tile([128, 128], BF16, name="ident_b")
        mask_bc = const.tile([128, L], F32, name="mask_bc")
        m1 = const.tile([1, L], F32, name="m1")

        ktp = octx.enter_context(tc.tile_pool(name="ktp", bufs=1))
        K_T = [ktp.tile([128, L], BF16, tag=f"k{h}", name=f"kT{h}")
               for h in range(NH)]
        vp = octx.enter_context(tc.tile_pool(name="vp", bufs=1))
        V = [vp.tile([128, NH * 129], BF16, tag=f"v{t}", name=f"vS{t}")
             for t in range(NKT)]
        wqt_pool = octx.enter_context(tc.tile_pool(name="wqt", bufs=1))
        wq_t = [wqt_pool.tile([128, JB], BF16, tag=f"wq{dc}", name=f"wqT{dc}")
                for dc in range(NDC)]
        xst = octx.enter_context(tc.tile_pool(name="xst", bufs=1))
        xbp = octx.enter_context(tc.tile_pool(name="xbp", bufs=2))
        xtp = octx.enter_context(tc.tile_pool(name="xtp", bufs=2))

        # identities (DVE) — emitted first so DVE is free for W drains later
        masks.make_identity(nc, identf[:])
        nc.vector.tensor_copy(ident_r[:], identf[:])
        nc.vector.tensor_copy(ident_b[:], identf[:])

        stage = {}

        def stage_dma(x_dram, ltile, key, conv):
            """DMA one [128, DIM] l-tile (f32r), convert to bf16 at once."""
            xs = xst.tile([128, DIM], F32R, tag=f"x{ltile % 2}",
                          name=f"x{ltile % 2}")
            nc.sync.dma_start(xs[:], x_dram[ltile * 128:(ltile + 1) * 128, :])
            xb = xbp.tile([128, DIM], BF16, tag=f"xb{ltile % 4}",
                          name=f"xb{ltile % 4}")
            if conv == "pool":
                nc.gpsimd.tensor_copy(xb[:], xs[:])
            else:
                nc.vector.tensor_copy(xb[:], xs[:])
            stage[key] = xb

        def transpose_x(xb_pair, pps, dr_cycle):
            """Transpose two converted l-tiles into 16 xt tiles [128, LB],
            two dc per PSUM tile to halve allocation pressure."""
            xt = []
            for dp in range(NDC // 2):
                pt = pps.tile([128, 2 * LB], BF16, tag="tp", name="tp")
                for j in range(2):
                    dc = 2 * dp + j
                    for lt in range(2):
                        nc.tensor.transpose(
                            pt[:, j * LB + lt * 128:j * LB + (lt + 1) * 128],
                            xb_pair[lt][:, dc * 128:(dc + 1) * 128],
                            ident_b[:])
                for j in range(2):
                    dc = 2 * dp + j
                    t = xtp.tile([128, LB], BF16, tag=f"xt{dc}",
                                 name=f"xt{dc}")
                    drain(nc, next(dr_cycle), t[:],
                          pt[:, j * LB:(j + 1) * LB])
                    xt.append(t)
            return xt

        # stage lb0 of xv right away (sync queue; DVE converts since the
        # Pool queue is saturated issuing W DMAs at startup)
        for tl in range(2):
            stage_dma(xv, tl, tl, "dve")
        nc.sync.dma_start(m1[:], msk[:, :])

        # ---- phases 0+1 scope: wk_t/wv_t ----
        with tc.tile_pool(name="wkvt", bufs=1) as wkvt_pool:
            wk_t = [wkvt_pool.tile([128, JB], BF16, tag=f"wk{dc}",
                                   name=f"wkT{dc}") for dc in range(NDC)]
            wv_t = [wkvt_pool.tile([128, JB], BF16, tag=f"wv{dc}",
                                   name=f"wvT{dc}") for dc in range(NDC)]
            wdr = it.cycle(["dve", "dve", "act"])
            # phase 0: Wk/Wv halves in, transpose (f32r: 1.5 cyc/row)
            with tc.tile_pool(name="wst", bufs=1) as wst, \
                 tc.tile_pool(name="wps", bufs=4, space="PSUM") as wps:
                engines = (nc.scalar, nc.gpsimd)
                for w_dram, dst, nm in ((wk, wk_t, "wk"), (wv, wv_t, "wv")):
                    rows = [[None, None] for _ in range(4)]
                    for hf in range(2):      # all first halves land first
                        for i in range(4):
                            r = wst.tile([128, HD], F32R, tag=f"w{i}h{hf}",
                                         name=f"{nm}r{i}h{hf}")
                            engines[i % 2].dma_start(
                                r[:], w_dram[i * 128:(i + 1) * 128,
                                             hf * HD:(hf + 1) * HD])
                            rows[i][hf] = r
                    for dc in range(NDC):
                        hf, dco = divmod(dc, NDC // 2)
                        pt = wps.tile([128, JB], F32R, tag="wp", name="wp")
                        for i in range(4):
                            nc.tensor.transpose(
                                pt[:, i * 128:(i + 1) * 128],
                                rows[i][hf][:, dco * 128:(dco + 1) * 128],
                                ident_r[:])
                        drain(nc, next(wdr), dst[dc][:], pt[:])

            # ---- phase 1: K/V projection (+ Wq^T after lb0) ----
            with tc.tile_pool(name="wqst", bufs=2) as wqst, \
                 tc.tile_pool(name="p1ps", bufs=2, space="PSUM") as pps, \
                 tc.tile_pool(name="p1ka", bufs=2, space="PSUM") as kps, \
                 tc.tile_pool(name="p1va", bufs=2, space="PSUM") as vps:
                dr_xt1 = it.cycle(["act", "act", "dve"])
                for lb in range(NLB):
                    if lb == 0:
                        for tl in range(2, 4):
                            stage_dma(xv, tl, tl, "dve")
                    elif lb == 1:
                        # mask broadcast (gpsimd) — needed only in phase 2,
                        # deferred so it doesn't block startup converts
                        nc.gpsimd.partition_broadcast(mask_bc[:], m1[:])
                    xt = transpose_x(
                        [stage.pop(2 * lb), stage.pop(2 * lb + 1)],
                        pps, dr_xt1)
                    # K: 4 head-chunks chained into one 2-bank PSUM tile
                    ka = kps.tile([128, 4 * LB], F32, tag="ka", name="ka")
                    for jt in range(NH):
                        for dc in range(NDC):
                            nc.tensor.matmul(
                                ka[:, jt * LB:(jt + 1) * LB],
                                wk_t[dc][:, jt * 128:(jt + 1) * 128],
                                xt[dc][:],
                                start=(dc == 0), stop=(dc == NDC - 1))
                    for jt in range(NH):
                        drain(nc, "dve", K_T[jt][:, lb * LB:(lb + 1) * LB],
                              ka[:, jt * LB:(jt + 1) * LB])
                    if lb == 0:
                        # V ones columns (DVE; must precede lb0's V drains)
                        for t in range(NKT):
                            nc.vector.memset(V[t][:], 1.0)
                        # stage Wq rows (gpsimd queue; Act's queue must stay
                        # clear for drains)
                        wq_rows = []
                        for i in range(4):
                            r = wqst.tile([128, DIM], F32R, tag=f"q{i % 2}",
                                          name=f"wqr{i}")
                            nc.gpsimd.dma_start(
                                r[:], wq[i * 128:(i + 1) * 128, :])
                            wq_rows.append(r)
                    # V: seq-major with ones column per head
                    for lt in range(2):
                        va = vps.tile([128, JB], F32, tag="va", name="va")
                        for dc in range(NDC):
                            nc.tensor.matmul(
                                va[:], xt[dc][:, lt * 128:(lt + 1) * 128],
                                wv_t[dc][:],
                                start=(dc == 0), stop=(dc == NDC - 1))
                        kt = lb * 2 + lt
                        for h in range(NH):
                            drain(nc, "dve",
                                  V[kt][:, h * 129:h * 129 + 128],
                                  va[:, h * 128:(h + 1) * 128])
                    if lb == 0:
                        # Wq^T between lb0 and lb1, PSUM via the "va" tag
                        for dc in range(NDC):
                            pt = vps.tile([128, JB], F32R, tag="va",
                                          name="wqp")
                            for i in range(4):
                                nc.tensor.transpose(
                                    pt[:, i * 128:(i + 1) * 128],
                                    wq_rows[i][:, dc * 128:(dc + 1) * 128],
                                    ident_r[:])
                            drain(nc, next(wdr), wq_t[dc][:], pt[:])
                    # prefetch next lb (xv); tail feeds phase-2 qb0 (xq)
                    if lb < NLB - 1:
                        for lt in range(2):
                            tl = 2 * (lb + 1) + lt
                            if tl >= 4:
                                stage_dma(xv, tl, tl, "pool")
                    if lb == NLB - 2:
                        for lt in range(2):
                            stage_dma(xq, lt, 100 + lt, "pool")
                    elif lb == NLB - 1:
                        for lt in range(2):
                            stage_dma(xq, 2 + lt, 102 + lt, "pool")

        # ---- phase 2: Q projection pipelined with attention ----
        with tc.tile_pool(name="qtp", bufs=2) as qtp, \
             tc.tile_pool(name="et", bufs=3) as etp, \
             tc.tile_pool(name="ot", bufs=1) as otp, \
             tc.tile_pool(name="rsb", bufs=4) as rsb, \
             tc.tile_pool(name="p2tp", bufs=2, space="PSUM") as pps2, \
             tc.tile_pool(name="p2s", bufs=2, space="PSUM") as sps, \
             tc.tile_pool(name="p2a", bufs=1, space="PSUM") as aps:
            dr_xt2 = it.cycle(["dve"])
            QT = {}           # qb -> 4 tiles [128, QB]
            ots = {}          # qb -> 4 tiles [128, JB]
            ET = {}           # (qb, h) -> 8 et tiles

            def qproj_chunk(nqb, ci):
                """Emit chunk ci (0..3) of Q-projection for block nqb
                (monolithic form, used by the prologue)."""
                for piece in qproj_pieces(nqb, ci):
                    piece()

            def qproj_pieces(nqb, ci):
                """Chunk ci (0..3) of Q-projection for nqb as 8 closures of
                ~k2-sized PE work, for interleaving into the exp-paced
                attention loop."""
                if ci == 0:
                    QT[nqb] = [qtp.tile([128, QB], BF16, tag=f"q{h}",
                                        name=f"qT{h}") for h in range(NH)]
                if ci in (0, 2):
                    i = ci // 2
                    pair = [stage.pop(100 + 4 * nqb + 2 * i + lt)
                            for lt in range(2)]
                    xt = stage[(nqb, i)] = []

                    def tp_piece(p):
                        pt = pps2.tile([128, 2 * LB], BF16, tag="tp",
                                       name="tp")
                        for j in range(2):
                            dc = 2 * p + j
                            for lt in range(2):
                                nc.tensor.transpose(
                                    pt[:, j * LB + lt * 128:
                                       j * LB + (lt + 1) * 128],
                                    pair[lt][:, dc * 128:(dc + 1) * 128],
                                    ident_b[:])
                        for j in range(2):
                            dc = 2 * p + j
                            t = xtp.tile([128, LB], BF16, tag=f"xt{dc}",
                                         name=f"xt{dc}")
                            drain(nc, next(dr_xt2), t[:],
                                  pt[:, j * LB:(j + 1) * LB])
                            xt.append(t)
                    return [lambda p=p: tp_piece(p) for p in range(8)]
                i = (ci - 1) // 2
                xt = stage[(nqb, i)]
                box = {}

                def mm_piece(p):
                    jt, hf = divmod(p, 2)
                    if p == 0:
                        box[0] = sps.tile([128, 4 * LB], F32, tag="s",
                                          name="qa")
                        stage.pop((nqb, i), None)
                    qa = box[0]
                    col = jt * LB
                    for dc in range(hf * 8, hf * 8 + 8):
                        nc.tensor.matmul(
                            qa[:, col:col + LB],
                            wq_t[dc][:, jt * 128:(jt + 1) * 128],
                            xt[dc][:],
                            start=(dc == 0), stop=(dc == NDC - 1))
                    if hf == 1:
                        nc.vector.tensor_mul(
                            QT[nqb][jt][:, i * LB:(i + 1) * LB],
                            qa[:, col:col + LB],
                            mask_bc[:, nqb * QB + i * LB:
                                    nqb * QB + (i + 1) * LB])
                return [lambda p=p: mm_piece(p) for p in range(8)]

            def attn_chains(qb_h, a01, a23, k2):
                """8 attn*V matmuls for the pending head: chain qs=k2//2,
                half k2%2 — each qs chain spans 2 consecutive k2 steps so
                chains never interleave within a PSUM zero region. The qs
                drain is emitted as soon as its chain stops, freeing the
                PSUM bank for the next head without waiting for slot end."""
                pqb, ph = qb_h
                ets = ET[(pqb, ph)]
                qs, half = divmod(k2, 2)
                a = a01 if qs < 2 else a23
                base = 129 * (qs % 2)
                for kc in range(half * 8, half * 8 + 8):
                    nc.tensor.matmul(
                        a[:, base:base + 129],
                        ets[kc // 2][:, (kc % 2) * 512
                                     + qs * 128:(kc % 2) * 512
                                     + (qs + 1) * 128],
                        V[kc][:, ph * 129:(ph + 1) * 129],
                        start=(kc == 0), stop=(kc == NKT - 1))
                if half == 1:
                    if pqb not in ots:
                        ots[pqb] = [otp.tile([128, JB], F32, tag=f"ot{q}",
                                             name=f"ot{q}") for q in range(4)]
                    rec = rsb.tile([128, 1], F32, tag="rec", name="rec")
                    nc.vector.reciprocal(rec[:], a[:, base + 128:base + 129])
                    nc.vector.tensor_scalar_mul(
                        ots[pqb][qs][:, ph * 128:(ph + 1) * 128],
                        a[:, base:base + 128], rec[:])
                    if ph == NH - 1:
                        q0 = pqb * QB + qs * 128
                        nc.sync.dma_start(out[q0:q0 + 128, :],
                                          ots[pqb][qs][:])

            def attn_drain(qb_h):
                """Slot-end bookkeeping."""
                del ET[qb_h]

            # prologue: stage qb1's tiles, project qb 0
            for t in range(4):
                stage_dma(xq, 4 + t, 104 + t, "pool")
            for ci in range(4):
                qproj_chunk(0, ci)

            # depth-2 chain pipeline: the attn*V chains for slot j run
            # during slot j+2, so qb3's exps precompute during its scores
            # slots and the tail is pure PE work (et bufs=3 covers the lag)
            pend_q = []
            for qb in range(NQB):
                for h in range(NH):
                    tgt = None
                    if len(pend_q) == 2:
                        tgt = pend_q.pop(0)
                        a01 = aps.tile([128, 258], F32, tag="a01", name="a01")
                        a23 = aps.tile([128, 258], F32, tag="a23", name="a23")
                    ET[(qb, h)] = [etp.tile([128, 1024], BF16, tag=f"et{k2}",
                                            name=f"et{k2}")
                                   for k2 in range(NKT // 2)]
                    pieces = (qproj_pieces(qb + 1, h)
                              if qb < NQB - 1 else [None] * 8)
                    for k2 in range(NKT // 2):
                        # chains and proj pieces first: both are ready to
                        # run, while the s-tile allocation below can stall
                        # on exp throughput
                        if tgt is not None:
                            attn_chains(tgt, a01, a23, k2)
                        if pieces[k2] is not None:
                            pieces[k2]()
                        s = sps.tile([128, 1024], F32, tag="s", name="s")
                        for i in range(2):
                            nc.tensor.matmul(
                                s[:, i * 512:(i + 1) * 512],
                                K_T[h][:, (2 * k2 + i) * 128:
                                       (2 * k2 + i + 1) * 128],
                                QT[qb][h][:],
                                start=True, stop=True)
                        nc.scalar.activation(
                            ET[(qb, h)][k2][:], s[:],
                            mybir.ActivationFunctionType.Exp,
                            scale=float(SCALE))
                    if tgt is not None:
                        attn_drain(tgt)
                    if h == NH - 1 and qb < NQB - 2:
                        for t in range(4):
                            tl = 4 * (qb + 2) + t
                            stage_dma(xq, tl, 100 + tl, "pool")
                    pend_q.append((qb, h))
            # tail: two pending chain sets, exps already done
            for tgt in pend_q:
                a01 = aps.tile([128, 258], F32, tag="a01", name="a01")
                a23 = aps.tile([128, 258], F32, tag="a23", name="a23")
                for k2 in range(NKT // 2):
                    attn_chains(tgt, a01, a23, k2)
                attn_drain(tgt)


_NC_CACHE = None


def _get_nc():
    global _NC_CACHE
    if _NC_CACHE is None:
        _NC_CACHE = build_nc()
    return _NC_CACHE


def make_in_maps(query_tensor, value_tensor, attention_mask, Wq, Wk, Wv):
    in_maps = []
    for c in range(N_CORES):
        b, g = divmod(c, 4)
        j0 = g * JB
        in_maps.append({
            "xq": np.ascontiguousarray(query_tensor[b], dtype=np.float32),
            "xv": np.ascontiguousarray(value_tensor[b], dtype=np.float32),
            "msk": np.ascontiguousarray(
                attention_mask[b].reshape(1, L), dtype=np.float32),
            "wq": np.ascontiguousarray(Wq[j0:j0 + JB], dtype=np.float32),
            "wk": np.ascontiguousarray(Wk[j0:j0 + JB], dtype=np.float32),
            "wv": np.ascontiguousarray(Wv[j0:j0 + JB], dtype=np.float32),
        })
    return in_maps


def assemble(results):
    out = np.empty((B, L, DIM), dtype=np.float32)
    for c in range(N_CORES):
        b, g = divmod(c, 4)
        out[b, :, g * JB:(g + 1) * JB] = results[c]["out"]
    return out


def kernel(query_tensor, value_tensor, attention_mask, Wq, Wk, Wv):
    nc = _get_nc()
    in_maps = make_in_maps(np.asarray(query_tensor), np.asarray(value_tensor),
                           np.asarray(attention_mask), np.asarray(Wq),
                           np.asarray(Wk), np.asarray(Wv))
    res = run_bass_kernel_spmd(nc, in_maps, core_ids=list(range(N_CORES)))
    return assemble(res.results)


# revision 49
# speedup vs baseline: 4.1418x; 2.9980x over previous
"""Multi-head attention (B=2, L=2048, DIM=2048, H=16) on 8 TRN2 NeuronCores.

Sharding: data-parallel over batch (2) x tensor-parallel over head groups (4).
Core c handles batch c//4, heads [4*(c%4), 4*(c%4)+4): it receives the full
query/value tensors for its batch plus the 512-row slices of Wq/Wk/Wv for its
heads, and produces the [2048, 512] slice of the output.

Schedule, designed so the PE never starves and the Act engine's exp work
(~133us) hides under projection matmuls:
  startup: xv staged immediately; x tiles are DMA'd f32r then converted to
           bf16 right away (bf16 PE transposes run 1 cyc/row).
  phase 0: Wk/Wv transposed up front (staged as halves so the first
           transpose starts ~3us in); Wq^T is emitted inside phase 1
           (reusing the V-projection PSUM tag) since it's needed much later.
  phase 1: K/V projection over 8 l-blocks; K accumulated 4-heads-per-tile.
  phase 2: per 512-query block, a software pipeline interleaved at k2
           (1k-score-tile) granularity: the pending head's attn*V chains
           and one piece of the NEXT block's Q projection are emitted
           inside the exp-paced score loop so the PE stays saturated while
           Act does exp. The query mask is folded into the Q drain via
           tensor_mul with a broadcast mask tile; softmax denominators come
           from a ones-column appended to V; per-qs drains and out-DMAs
           are emitted as soon as each accumulation chain stops.
PSUM drains only on Act/DVE (hardware forbids GPSIMD PSUM access); GpSimd
does bf16 converts and part of the DMA issue. Pool scopes nest LIFO for
the stack allocator. CoreSim cost model: ~347us; PE busy 91.5% (318us of
PE work at 2.4 GHz: 164us QKV matmuls, 110us scores+attn*V, 44us
transposes).
"""

import sys

for p in ("/opt/trn_rl_repo", "/opt/pypackages"):
    if p not in sys.path:
        sys.path.insert(0, p)

import contextlib
import itertools as it

import numpy as np

import concourse.bacc as bacc
import concourse.mybir as mybir
import concourse.tile as tile
from concourse import masks
from concourse.bass_utils import run_bass_kernel_spmd

N_CORES = 8
B, L, DIM, H = 2, 2048, 2048, 16
JB = DIM // 4          # 512 output features per core (4 heads)
DH = 128               # head dim
NH = 4                 # heads per core
NDC = DIM // 128       # 16 contraction chunks
LB = 256               # l-block for QKV staging
NLB = L // LB          # 8
NKT = L // 128         # 16 k tiles
QB = 512               # q-block for attention
NQB = L // QB          # 4
HD = DIM // 2
SCALE = 1.0 / np.sqrt(DH)

F32 = mybir.dt.float32
F32R = mybir.dt.float32r
BF16 = mybir.dt.bfloat16


def drain(nc, eng, out_ap, in_ap):
    """PSUM->SBUF copy on the named engine (act or dve only)."""
    if eng == "act":
        nc.scalar.copy(out_ap, in_ap)
    else:
        nc.vector.tensor_copy(out_ap, in_ap)


def build_nc():
    nc = bacc.Bacc("TRN2", target_bir_lowering=False, debug=False,
                   num_devices=N_CORES)
    xq = nc.dram_tensor("xq", [L, DIM], F32R, kind="ExternalInput").ap()
    xv = nc.dram_tensor("xv", [L, DIM], F32R, kind="ExternalInput").ap()
    msk = nc.dram_tensor("msk", [1, L], F32, kind="ExternalInput").ap()
    wq = nc.dram_tensor("wq", [JB, DIM], F32R, kind="ExternalInput").ap()
    wk = nc.dram_tensor("wk", [JB, DIM], F32R, kind="ExternalInput").ap()
    wv = nc.dram_tensor("wv", [JB, DIM], F32R, kind="ExternalInput").ap()
    out = nc.dram_tensor("out", [L, JB], F32, kind="ExternalOutput").ap()

    with tile.TileContext(nc) as tc:
        build_kernel(nc, tc, xq, xv, msk, wq, wk, wv, out)
    nc.compile()
    return nc


def build_kernel(nc, tc, xq, xv, msk, wq, wk, wv, out):
    octx = contextlib.ExitStack()
    with octx:
        # ---- persistent pools (live for the whole kernel) ----
        const = octx.enter_context(tc.tile_pool(name="const", bufs=1))
        identf = const.tile([128, 128], F32, name="identf")
        ident_r = const.tile([128, 128], F32R, name="ident_r")
        ident_b = const.tile([128, 128], BF16, name="ident_b")
        mask_bc = const.tile([128, L], F32, name="mask_bc")
        m1 = const.tile([1, L], F32, name="m1")

        ktp = octx.enter_context(tc.tile_pool(name="ktp", bufs=1))
        K_T = [ktp.tile([128, L], BF16, tag=f"k{h}", name=f"kT{h}")
               for h in range(NH)]
        vp = octx.enter_context(tc.tile_pool(name="vp", bufs=1))
        V = [vp.tile([128, NH * 129], BF16, tag=f"v{t}", name=f"vS{t}")
             for t in range(NKT)]
        wqt_pool = octx.enter_context(tc.tile_pool(name="wqt", bufs=1))
        wq_t = [wqt_pool.tile([128, JB], BF16, tag=f"wq{dc}", name=f"wqT{dc}")
                for dc in range(NDC)]
        xst = octx.enter_context(tc.tile_pool(name="xst", bufs=1))
        xbp = octx.enter_context(tc.tile_pool(name="xbp", bufs=2))
        xtp = octx.enter_context(tc.tile_pool(name="xtp", bufs=2))

        # identities (DVE) — emitted first so DVE is free for W drains later
        masks.make_identity(nc, identf[:])
        nc.vector.tensor_copy(ident_r[:], identf[:])
        nc.vector.tensor_copy(ident_b[:], identf[:])

        stage = {}

        def stage_dma(x_dram, ltile, key, conv):
            """DMA one [128, DIM] l-tile (f32r), convert to bf16 at once."""
            xs = xst.tile([128, DIM], F32R, tag=f"x{ltile % 2}",
                          name=f"x{ltile % 2}")
            nc.sync.dma_start(xs[:], x_dram[ltile * 128:(ltile + 1) * 128, :])
            xb = xbp.tile([128, DIM], BF16, tag=f"xb{ltile % 4}",
                          name=f"xb{ltile % 4}")
            if conv == "pool":
                nc.gpsimd.tensor_copy(xb[:], xs[:])
            else:
                nc.vector.tensor_copy(xb[:], xs[:])
            stage[key] = xb

        def transpose_x(xb_pair, pps, dr_cycle):
            """Transpose two converted l-tiles into 16 xt tiles [128, LB],
            two dc per PSUM tile to halve allocation pressure."""
            xt = []
            for dp in range(NDC // 2):
                pt = pps.tile([128, 2 * LB], BF16, tag="tp", name="tp")
                for j in range(2):
                    dc = 2 * dp + j
                    for lt in range(2):
                        nc.tensor.transpose(
                            pt[:, j * LB + lt * 128:j * LB + (lt + 1) * 128],
                            xb_pair[lt][:, dc * 128:(dc + 1) * 128],
                            ident_b[:])
                for j in range(2):
                    dc = 2 * dp + j
                    t = xtp.tile([128, LB], BF16, tag=f"xt{dc}",
                                 name=f"xt{dc}")
                    drain(nc, next(dr_cycle), t[:],
                          pt[:, j * LB:(j + 1) * LB])
                    xt.append(t)
            return xt

        # stage lb0 of xv right away (sync queue; DVE converts since the
        # Pool queue is saturated issuing W DMAs at startup)
        for tl in range(2):
            stage_dma(xv, tl, tl, "dve")
        nc.sync.dma_start(m1[:], msk[:, :])

        # ---- phases 0+1 scope: wk_t/wv_t ----
        with tc.tile_pool(name="wkvt", bufs=1) as wkvt_pool:
            wk_t = [wkvt_pool.tile([128, JB], BF16, tag=f"wk{dc}",
                                   name=f"wkT{dc}") for dc in range(NDC)]
            wv_t = [wkvt_pool.tile([128, JB], BF16, tag=f"wv{dc}",
                                   name=f"wvT{dc}") for dc in range(NDC)]
            wdr = it.cycle(["dve", "dve", "act"])
            # phase 0: Wk/Wv staged as quarter-tiles so the first transpose
            # can start ~2us in (f32r transposes: 1.5 cyc/row)
            QD = DIM // 4
            with tc.tile_pool(name="wst", bufs=1) as wst, \
                 tc.tile_pool(name="wps", bufs=6, space="PSUM") as wps:
                engines = (nc.scalar, nc.gpsimd)
                for w_dram, dst, nm in ((wk, wk_t, "wk"), (wv, wv_t, "wv")):
                    rows = [[None] * 4 for _ in range(4)]
                    for qf in range(4):      # first quarters land first
                        for i in range(4):
                            r = wst.tile([128, QD], F32R, tag=f"w{i}q{qf}",
                                         name=f"{nm}r{i}q{qf}")
                            engines[i % 2].dma_start(
                                r[:], w_dram[i * 128:(i + 1) * 128,
                                             qf * QD:(qf + 1) * QD])
                            rows[i][qf] = r
                    for dc in range(NDC):
                        qf, dco = divmod(dc, NDC // 4)
                        pt = wps.tile([128, JB], F32R, tag="wp", name="wp")
                        for i in range(4):
                            nc.tensor.transpose(
                                pt[:, i * 128:(i + 1) * 128],
                                rows[i][qf][:, dco * 128:(dco + 1) * 128],
                                ident_r[:])
                        drain(nc, next(wdr), dst[dc][:], pt[:])

            # ---- phase 1: K/V projection (+ Wq^T after lb0) ----
            with tc.tile_pool(name="wqst", bufs=2) as wqst, \
                 tc.tile_pool(name="p1ps", bufs=2, space="PSUM") as pps, \
                 tc.tile_pool(name="p1ka", bufs=2, space="PSUM") as kps, \
                 tc.tile_pool(name="p1va", bufs=2, space="PSUM") as vps:
                dr_xt1 = it.cycle(["act", "act", "dve"])
                for lb in range(NLB):
                    if lb == 0:
                        # lb1 converts on Pool (its DMA queue drains by then)
                        # so DVE stays free for the W^T drains gating K
                        for tl in range(2, 4):
                            stage_dma(xv, tl, tl, "pool")
                    elif lb == 1:
                        # mask broadcast (gpsimd) — needed only in phase 2,
                        # deferred so it doesn't block startup converts
                        nc.gpsimd.partition_broadcast(mask_bc[:], m1[:])
                    xt = transpose_x(
                        [stage.pop(2 * lb), stage.pop(2 * lb + 1)],
                        pps, dr_xt1)
                    # K: 4 head-chunks chained into one 2-bank PSUM tile
                    ka = kps.tile([128, 4 * LB], F32, tag="ka", name="ka")
                    for jt in range(NH):
                        for dc in range(NDC):
                            nc.tensor.matmul(
                                ka[:, jt * LB:(jt + 1) * LB],
                                wk_t[dc][:, jt * 128:(jt + 1) * 128],
                                xt[dc][:],
                                start=(dc == 0), stop=(dc == NDC - 1))
                    for jt in range(NH):
                        drain(nc, "dve", K_T[jt][:, lb * LB:(lb + 1) * LB],
                              ka[:, jt * LB:(jt + 1) * LB])
                    if lb == 0:
                        # V ones columns (DVE; must precede lb0's V drains)
                        for t in range(NKT):
                            nc.vector.memset(V[t][:], 1.0)
                        # stage Wq rows (gpsimd queue; Act's queue must stay
                        # clear for drains)
                        wq_rows = []
                        for i in range(4):
                            r = wqst.tile([128, DIM], F32R, tag=f"q{i % 2}",
                                          name=f"wqr{i}")
                            nc.gpsimd.dma_start(
                                r[:], wq[i * 128:(i + 1) * 128, :])
                            wq_rows.append(r)
                    # V: seq-major with ones column per head
                    for lt in range(2):
                        va = vps.tile([128, JB], F32, tag="va", name="va")
                        for dc in range(NDC):
                            nc.tensor.matmul(
                                va[:], xt[dc][:, lt * 128:(lt + 1) * 128],
                                wv_t[dc][:],
                                start=(dc == 0), stop=(dc == NDC - 1))
                        kt = lb * 2 + lt
                        for h in range(NH):
                            drain(nc, "dve",
                                  V[kt][:, h * 129:h * 129 + 128],
                                  va[:, h * 128:(h + 1) * 128])
                    if lb == 0:
                        # Wq^T between lb0 and lb1, PSUM via the "va" tag
                        for dc in range(NDC):
                            pt = vps.tile([128, JB], F32R, tag="va",
                                          name="wqp")
                            for i in range(4):
                                nc.tensor.transpose(
                                    pt[:, i * 128:(i + 1) * 128],
                                    wq_rows[i][:, dc * 128:(dc + 1) * 128],
                                    ident_r[:])
                            drain(nc, next(wdr), wq_t[dc][:], pt[:])
                    # prefetch next lb (xv); tail feeds phase-2 qb0 (xq)
                    if lb < NLB - 1:
                        for lt in range(2):
                            tl = 2 * (lb + 1) + lt
                            if tl >= 4:
                                stage_dma(xv, tl, tl, "pool")
                    if lb == NLB - 2:
                        for lt in range(2):
                            stage_dma(xq, lt, 100 + lt, "pool")
                    elif lb == NLB - 1:
                        for lt in range(2):
                            stage_dma(xq, 2 + lt, 102 + lt, "pool")

        # ---- phase 2: Q projection pipelined with attention ----
        with tc.tile_pool(name="qtp", bufs=2) as qtp, \
             tc.tile_pool(name="et", bufs=3) as etp, \
             tc.tile_pool(name="ot", bufs=1) as otp, \
             tc.tile_pool(name="rsb", bufs=4) as rsb, \
             tc.tile_pool(name="p2tp", bufs=2, space="PSUM") as pps2, \
             tc.tile_pool(name="p2s", bufs=2, space="PSUM") as sps, \
             tc.tile_pool(name="p2a", bufs=1, space="PSUM") as aps:
            dr_xt2 = it.cycle(["dve"])
            QT = {}           # qb -> 4 tiles [128, QB]
            ots = {}          # qb -> 4 tiles [128, JB]
            ET = {}           # (qb, h) -> 8 et tiles

            def qproj_chunk(nqb, ci):
                """Emit chunk ci (0..3) of Q-projection for block nqb
                (monolithic form, used by the prologue)."""
                for piece in qproj_pieces(nqb, ci):
                    piece()

            def qproj_pieces(nqb, ci):
                """Chunk ci (0..3) of Q-projection for nqb as 8 closures of
                ~k2-sized PE work, for interleaving into the exp-paced
                attention loop."""
                if ci == 0:
                    QT[nqb] = [qtp.tile([128, QB], BF16, tag=f"q{h}",
                                        name=f"qT{h}") for h in range(NH)]
                if ci in (0, 2):
                    i = ci // 2
                    pair = [stage.pop(100 + 4 * nqb + 2 * i + lt)
                            for lt in range(2)]
                    xt = stage[(nqb, i)] = []

                    def tp_piece(p):
                        pt = pps2.tile([128, 2 * LB], BF16, tag="tp",
                                       name="tp")
                        for j in range(2):
                            dc = 2 * p + j
                            for lt in range(2):
                                nc.tensor.transpose(
                                    pt[:, j * LB + lt * 128:
                                       j * LB + (lt + 1) * 128],
                                    pair[lt][:, dc * 128:(dc + 1) * 128],
                                    ident_b[:])
                        for j in range(2):
                            dc = 2 * p + j
                            t = xtp.tile([128, LB], BF16, tag=f"xt{dc}",
                                         name=f"xt{dc}")
                            drain(nc, next(dr_xt2), t[:],
                                  pt[:, j * LB:(j + 1) * LB])
                            xt.append(t)
                    return [lambda p=p: tp_piece(p) for p in range(8)]
                i = (ci - 1) // 2
                xt = stage[(nqb, i)]
                box = {}

                def mm_piece(p):
                    jt, hf = divmod(p, 2)
                    if p == 0:
                        box[0] = sps.tile([128, 4 * LB], F32, tag="s",
                                          name="qa")
                        stage.pop((nqb, i), None)
                    qa = box[0]
                    col = jt * LB
                    for dc in range(hf * 8, hf * 8 + 8):
                        nc.tensor.matmul(
                            qa[:, col:col + LB],
                            wq_t[dc][:, jt * 128:(jt + 1) * 128],
                            xt[dc][:],
                            start=(dc == 0), stop=(dc == NDC - 1))
                    if hf == 1:
                        nc.vector.tensor_mul(
                            QT[nqb][jt][:, i * LB:(i + 1) * LB],
                            qa[:, col:col + LB],
                            mask_bc[:, nqb * QB + i * LB:
                                    nqb * QB + (i + 1) * LB])
                return [lambda p=p: mm_piece(p) for p in range(8)]

            def attn_chains(qb_h, a01, a23, k2):
                """8 attn*V matmuls for the pending head: chain qs=k2//2,
                half k2%2 — each qs chain spans 2 consecutive k2 steps so
                chains never interleave within a PSUM zero region. The qs
                drain is emitted as soon as its chain stops, freeing the
                PSUM bank for the next head without waiting for slot end."""
                pqb, ph = qb_h
                ets = ET[(pqb, ph)]
                qs, half = divmod(k2, 2)
                a = a01 if qs < 2 else a23
                base = 129 * (qs % 2)
                for kc in range(half * 8, half * 8 + 8):
                    nc.tensor.matmul(
                        a[:, base:base + 129],
                        ets[kc // 2][:, (kc % 2) * 512
                                     + qs * 128:(kc % 2) * 512
                                     + (qs + 1) * 128],
                        V[kc][:, ph * 129:(ph + 1) * 129],
                        start=(kc == 0), stop=(kc == NKT - 1))
                if half == 1:
                    if pqb not in ots:
                        ots[pqb] = [otp.tile([128, JB], F32, tag=f"ot{q}",
                                             name=f"ot{q}") for q in range(4)]
                    rec = rsb.tile([128, 1], F32, tag="rec", name="rec")
                    nc.vector.reciprocal(rec[:], a[:, base + 128:base + 129])
                    nc.vector.tensor_scalar_mul(
                        ots[pqb][qs][:, ph * 128:(ph + 1) * 128],
                        a[:, base:base + 128], rec[:])
                    if ph == NH - 1:
                        q0 = pqb * QB + qs * 128
                        nc.sync.dma_start(out[q0:q0 + 128, :],
                                          ots[pqb][qs][:])

            def attn_drain(qb_h):
                """Slot-end bookkeeping."""
                del ET[qb_h]

            # prologue: stage qb1's tiles, project qb 0
            for t in range(4):
                stage_dma(xq, 4 + t, 104 + t, "pool")
            for ci in range(4):
                qproj_chunk(0, ci)

            # depth-2 chain pipeline: the attn*V chains for slot j run
            # during slot j+2, so qb3's exps precompute during its scores
            # slots and the tail is pure PE work (et bufs=3 covers the lag)
            pend_q = []
            for qb in range(NQB):
                for h in range(NH):
                    tgt = None
                    if len(pend_q) == 2:
                        tgt = pend_q.pop(0)
                        a01 = aps.tile([128, 258], F32, tag="a01", name="a01")
                        a23 = aps.tile([128, 258], F32, tag="a23", name="a23")
                    ET[(qb, h)] = [etp.tile([128, 1024], BF16, tag=f"et{k2}",
                                            name=f"et{k2}")
                                   for k2 in range(NKT // 2)]
                    pieces = (qproj_pieces(qb + 1, h)
                              if qb < NQB - 1 else [None] * 8)
                    for k2 in range(NKT // 2):
                        # chains and proj pieces first: both are ready to
                        # run, while the s-tile allocation below can stall
                        # on exp throughput
                        if tgt is not None:
                            attn_chains(tgt, a01, a23, k2)
                        if pieces[k2] is not None:
                            pieces[k2]()
                        s = sps.tile([128, 1024], F32, tag="s", name="s")
                        for i in range(2):
                            nc.tensor.matmul(
                                s[:, i * 512:(i + 1) * 512],
                                K_T[h][:, (2 * k2 + i) * 128:
                                       (2 * k2 + i + 1) * 128],
                                QT[qb][h][:],
                                start=True, stop=True)
                        nc.scalar.activation(
                            ET[(qb, h)][k2][:], s[:],
                            mybir.ActivationFunctionType.Exp,
                            scale=float(SCALE))
                    if tgt is not None:
                        attn_drain(tgt)
                    if h == NH - 1 and qb < NQB - 2:
                        for t in range(4):
                            tl = 4 * (qb + 2) + t
                            stage_dma(xq, tl, 100 + tl, "pool")
                    pend_q.append((qb, h))
            # tail: two pending chain sets
            tgt = pend_q[0]
            a01 = aps.tile([128, 258], F32, tag="a01", name="a01")
            a23 = aps.tile([128, 258], F32, tag="a23", name="a23")
            for k2 in range(NKT // 2):
                attn_chains(tgt, a01, a23, k2)
            attn_drain(tgt)
            # final head: its exps are still landing, so spread chains by
            # dependency — each qs chain gets its own PSUM bank (the score
            # pool is free now) and all half-0 chunks (needing only
            # et[0..3]) run before any half-1 chunk
            tgt = pend_q[1]
            pqb, ph = tgt
            ets = ET[tgt]
            a01 = aps.tile([128, 258], F32, tag="a01", name="a01")
            a23 = aps.tile([128, 258], F32, tag="a23", name="a23")
            sx = sps.tile([128, 1024], F32, tag="s", name="atail")
            cmap = {0: (a01, 0), 1: (a23, 0), 2: (sx, 0), 3: (sx, 512)}
            for half in range(2):
                for qs in range(4):
                    a, base = cmap[qs]
                    for kc in range(half * 8, half * 8 + 8):
                        nc.tensor.matmul(
                            a[:, base:base + 129],
                            ets[kc // 2][:, (kc % 2) * 512
                                         + qs * 128:(kc % 2) * 512
                                         + (qs + 1) * 128],
                            V[kc][:, ph * 129:(ph + 1) * 129],
                            start=(kc == 0), stop=(kc == NKT - 1))
                    if half == 1:
                        rec = rsb.tile([128, 1], F32, tag="rec", name="rec")
                        nc.vector.reciprocal(
                            rec[:], a[:, base + 128:base + 129])
                        nc.vector.tensor_scalar_mul(
                            ots[pqb][qs][:, ph * 128:(ph + 1) * 128],
                            a[:, base:base + 128], rec[:])
                        q0 = pqb * QB + qs * 128
                        nc.sync.dma_start(out[q0:q0 + 128, :],
                                          ots[pqb][qs][:])
            attn_drain(tgt)


_NC_CACHE = None


def _get_nc():
    global _NC_CACHE
    if _NC_CACHE is None:
        _NC_CACHE = build_nc()
    return _NC_CACHE


def make_in_maps(query_tensor, value_tensor, attention_mask, Wq, Wk, Wv):
    in_maps = []
    for c in range(N_CORES):
        b, g = divmod(c, 4)
        j0 = g * JB
        in_maps.append({
            "xq": np.ascontiguousarray(query_tensor[b], dtype=np.float32),
            "xv": np.ascontiguousarray(value_tensor[b], dtype=np.float32),
            "msk": np.ascontiguousarray(
                attention_mask[b].reshape(1, L), dtype=np.float32),
            "wq": np.ascontiguousarray(Wq[j0:j0 + JB], dtype=np.float32),
            "wk": np.ascontiguousarray(Wk[j0:j0 + JB], dtype=np.float32),
            "wv": np.ascontiguousarray(Wv[j0:j0 + JB], dtype=np.float32),
        })
    return in_maps


def assemble(results):
    out = np.empty((B, L, DIM), dtype=np.float32)
    for c in range(N_CORES):
        b, g = divmod(c, 4)
        out[b, :, g * JB:(g + 1) * JB] = results[c]["out"]
    return out


def kernel(query_tensor, value_tensor, attention_mask, Wq, Wk, Wv):
    nc = _get_nc()
    in_maps = make_in_maps(np.asarray(query_tensor), np.asarray(value_tensor),
                           np.asarray(attention_mask), np.asarray(Wq),
                           np.asarray(Wk), np.asarray(Wv))
    res = run_bass_kernel_spmd(nc, in_maps, core_ids=list(range(N_CORES)))
    return assemble(res.results)
